# revision 1
# baseline (speedup 1.0000x reference)
"""AttnGraphPooling Trainium2 kernel v2 (8 NeuronCores, SPMD).

Key idea vs v1: lane-packing. Host packs each graph's nodes into "lanes"
of depth m (PACK): a lane holds up to m nodes of ONE graph, spread across
the m chunks of a group. Per group (m chunks x 128 lanes), the chunk
[E|VE] tiles are pre-accumulated across the m chunks with cheap vector
adds (valid because lane i has the same graph in every chunk), so the
expensive one-hot segment matmul runs once per GROUP instead of once per
chunk: PE matmuls drop from 3 to 2+1/m per chunk.

Padding slots use f_pad = Wk^{-1} @ (-25*ones): their attn row is ~-25 so
exp -> 0 in fp16, killing their contribution for free (no masks). Pad
lanes additionally get an all-zero one-hot row.

Dataflow per chunk pair: proj matmuls (fp16) -> PSUM; ACT exp -> st tile;
DVE mul val*E -> st; pair-sum st0+st1 -> pt (DVE/GpSimd alternating);
tree-add pts -> group acc (fp16); one seg matmul per group accumulates
oh^T @ acc into the block's PSUM. Epilogue is ACT-table-free (Newton
rsqrt) so the Exp table never swaps.
"""

import os as _os

import numpy as np

import concourse.bass as bass
import concourse.mybir as mybir
import concourse.tile as tile
from concourse.bass_utils import run_bass_kernel_spmd

N_CORES = 8
D = 256
GBLK = 128
BPC = 4  # blocks (of 128 graphs) per core
PACK = int(_os.environ.get("BASS_KERNEL_PACK", "4"))
SPLIT_MUL = _os.environ.get("BASS_KERNEL_SPLITMUL", "0") == "1"
FT_GROUPS_PER_DMA = 1

EPS_SOFTMAX = 1e-7
EPS_LN = 1e-5
PAD_ATT = -25.0

LAST_EXEC_TIME_NS = None
LAST_RESULTS = None
_nc_cache = {}


def _split_waits(nc, maxw=1):
    cnt = 0
    for f in nc.m.functions:
        for bb in f.blocks:
            newinsts = []
            for inst in bb.instructions:
                si = getattr(inst, "sync_info", None)
                if si is not None and si.on_wait and len(si.on_wait) > maxw:
                    waits = list(si.on_wait)
                    excess = waits[:-maxw]
                    si.on_wait = waits[-maxw:]
                    for i in range(0, len(excess), maxw):
                        nop = mybir.InstNoOp(
                            name=f"Wsplit-{cnt}",
                            engine=inst.engine,
                            bass_nofuse=True,
                            sync_info=mybir.SyncInfo(
                                on_wait=excess[i : i + maxw], on_update=[]
                            ),
                        )
                        cnt += 1
                        newinsts.append(nop)
                newinsts.append(inst)
            bb.instructions = newinsts
    return cnt


def _build_nc2(gpbs, m):
    """gpbs: tuple of groups-per-block for the BPC block slots (same on
    every core); m: lane depth (chunks per group)."""
    from contextlib import ExitStack

    R = mybir.dt.float16
    F32 = mybir.dt.float32
    TG = sum(gpbs)
    ncols = TG * m * 128
    assert m % 2 == 0

    nc = bass.Bass()
    fT_d = nc.dram_tensor("fT", [2, 128, ncols], R, kind="ExternalInput")
    oh_d = nc.dram_tensor("oh", [TG, 128, GBLK], R, kind="ExternalInput")
    wcat_d = nc.dram_tensor("wcat", [2, 128, 2 * D], R, kind="ExternalInput")
    vb_d = nc.dram_tensor("vbrep", [128, D], F32, kind="ExternalInput")
    epsd_d = nc.dram_tensor("epsrep", [128, D], F32, kind="ExternalInput")
    gm_d = nc.dram_tensor("gammarep", [128, D], F32, kind="ExternalInput")
    bt_d = nc.dram_tensor("betarep", [128, D], F32, kind="ExternalInput")
    y_d = nc.dram_tensor("y", [BPC * GBLK, D], F32, kind="ExternalOutput")

    with tile.TileContext(nc) as tc, ExitStack() as ctx:
        const = ctx.enter_context(tc.tile_pool(name="const", bufs=1))
        ftp = ctx.enter_context(tc.tile_pool(name="ft", bufs=6))
        ohp = ctx.enter_context(tc.tile_pool(name="oh", bufs=3))
        stp = ctx.enter_context(tc.tile_pool(name="st", bufs=8))
        vtp = ctx.enter_context(tc.tile_pool(name="vt", bufs=3))
        ptp = ctx.enter_context(tc.tile_pool(name="pt", bufs=10))
        accp = ctx.enter_context(tc.tile_pool(name="acc", bufs=4))
        epi = ctx.enter_context(tc.tile_pool(name="epi", bufs=2))
        pp_pool = ctx.enter_context(tc.tile_pool(name="pp", bufs=3, space="PSUM"))
        seg_pool = ctx.enter_context(tc.tile_pool(name="seg", bufs=2, space="PSUM"))

        # first fT tiles first so the PE starts ASAP
        gw = m * 128
        ft0_first = ftp.tile([128, gw], R, tag="ft0")
        nc.sync.dma_start(ft0_first[:], fT_d[0, :, 0:gw])
        ft1_first = ftp.tile([128, gw], R, tag="ft1")
        nc.sync.dma_start(ft1_first[:], fT_d[1, :, 0:gw])
        w0 = const.tile([128, 2 * D], R, tag="w0")
        nc.sync.dma_start(w0[:], wcat_d[0])
        w1 = const.tile([128, 2 * D], R, tag="w1")
        nc.sync.dma_start(w1[:], wcat_d[1])
        vb = const.tile([128, D], F32, tag="vb")
        nc.sync.dma_start(vb[:], vb_d[:])
        epsd = const.tile([128, D], F32, tag="epsd")
        nc.sync.dma_start(epsd[:], epsd_d[:])
        gm = const.tile([128, D], F32, tag="gm")
        nc.sync.dma_start(gm[:], gm_d[:])
        bt = const.tile([128, D], F32, tag="bt")
        nc.sync.dma_start(bt[:], bt_d[:])
        epsln = const.tile([128, 1], F32, tag="epsln")
        nc.gpsimd.memset(epsln[:], float(EPS_LN))
        magic = const.tile([128, 1], mybir.dt.uint32, tag="magic")
        nc.gpsimd.memset(magic[:], 0x5F3759DF)

        warm = const.tile([128, 1], F32, tag="warm")
        warm2 = const.tile([128, 1], F32, tag="warm2")
        nc.gpsimd.memset(warm[:], 1.0)
        nc.scalar.activation(warm2[:], warm[:], mybir.ActivationFunctionType.Exp)

        seg_tiles = {}
        pending_seg = []

        def emit_seg(items):
            for blk, oht, acct, start, stop in items:
                nc.tensor.matmul(
                    seg_tiles[blk][:],
                    oht[:],
                    acct[:],
                    start=start,
                    stop=stop,
                    skip_group_check=True,
                )
                if stop:
                    emit_epilogue(blk)

        def emit_epilogue(blk):
            seg_ps = seg_tiles.pop(blk)
            segc = epi.tile([128, 2 * D], F32, tag="segc")
            nc.scalar.copy(segc[:], seg_ps[:])
            segE = segc[:, 0:D]
            segVE = segc[:, D : 2 * D]
            den = epi.tile([128, D], F32, tag="den")
            nc.gpsimd.tensor_add(den[:], segE, epsd[:])
            rec = epi.tile([128, D], F32, tag="rec")
            nc.vector.reciprocal(rec[:], den[:])
            nvb = epi.tile([128, D], F32, tag="nvb")
            nc.gpsimd.tensor_mul(nvb[:], segE, vb[:])
            num = epi.tile([128, D], F32, tag="num")
            nc.gpsimd.tensor_add(num[:], segVE, nvb[:])
            fg = epi.tile([128, D], F32, tag="fg")
            ms = epi.tile([128, 1], F32, tag="ms")
            nc.vector.scalar_tensor_tensor(
                fg[:], num[:], 1.0, rec[:],
                op0=mybir.AluOpType.mult, op1=mybir.AluOpType.mult,
                accum_out=ms[:],
            )
            mean = epi.tile([128, 1], F32, tag="mean")
            nc.vector.tensor_scalar_mul(mean[:], ms[:], 1.0 / D)
            xm = epi.tile([128, D], F32, tag="xm")
            nc.vector.tensor_scalar_sub(xm[:], fg[:], mean[:])
            sq = epi.tile([128, D], F32, tag="sq")
            vs = epi.tile([128, 1], F32, tag="vs")
            nc.vector.scalar_tensor_tensor(
                sq[:], xm[:], 1.0, xm[:],
                op0=mybir.AluOpType.mult, op1=mybir.AluOpType.mult,
                accum_out=vs[:],
            )
            tt = epi.tile([128, 1], F32, tag="tt")
            nc.vector.scalar_tensor_tensor(
                tt[:], vs[:], 1.0 / D, epsln[:],
                op0=mybir.AluOpType.mult, op1=mybir.AluOpType.add,
            )
            hh = epi.tile([128, 1], mybir.dt.uint32, tag="hh")
            nc.vector.tensor_scalar(
                hh[:], tt[:].bitcast(mybir.dt.uint32), 1, None,
                op0=mybir.AluOpType.logical_shift_right,
            )
            yy = epi.tile([128, 1], mybir.dt.uint32, tag="yy")
            nc.vector.tensor_tensor(
                yy[:], magic[:], hh[:], op=mybir.AluOpType.subtract
            )
            rs = yy[:].bitcast(F32)
            for _ in range(3):
                y2 = epi.tile([128, 1], F32, tag="y2")
                nc.vector.tensor_tensor(y2[:], rs, rs, op=mybir.AluOpType.mult)
                hty = epi.tile([128, 1], F32, tag="hty")
                nc.vector.scalar_tensor_tensor(
                    hty[:], y2[:], -0.5, tt[:],
                    op0=mybir.AluOpType.mult, op1=mybir.AluOpType.mult,
                )
                cc = epi.tile([128, 1], F32, tag="cc")
                nc.vector.tensor_scalar_add(cc[:], hty[:], 1.5)
                ny = epi.tile([128, 1], F32, tag="ny")
                nc.vector.tensor_scalar_mul(ny[:], rs, cc[:])
                rs = ny[:]
            o1 = epi.tile([128, D], F32, tag="o1")
            nc.vector.tensor_scalar_mul(o1[:], xm[:], rs)
            o2 = epi.tile([128, D], F32, tag="o2")
            nc.gpsimd.tensor_mul(o2[:], o1[:], gm[:])
            oo = epi.tile([128, D], F32, tag="oo")
            nc.gpsimd.tensor_add(oo[:], o2[:], bt[:])
            nc.sync.dma_start(y_d[blk * GBLK : (blk + 1) * GBLK, :], oo[:])

        tg = 0
        for blk in range(BPC):
            for t in range(gpbs[blk]):
                colbase = tg * m * 128
                if tg == 0:
                    ft0, ft1 = ft0_first, ft1_first
                else:
                    ft0 = ftp.tile([128, gw], R, tag="ft0")
                    nc.sync.dma_start(ft0[:], fT_d[0, :, colbase : colbase + gw])
                    ft1 = ftp.tile([128, gw], R, tag="ft1")
                    nc.sync.dma_start(ft1[:], fT_d[1, :, colbase : colbase + gw])
                oht = ohp.tile([128, GBLK], R, tag="oh")
                nc.sync.dma_start(oht[:], oh_d[tg])
                if t == 0:
                    seg_tiles[blk] = seg_pool.tile(
                        [128, 2 * D], F32, name="seg", tag="seg"
                    )

                pts = []
                pp2 = st = None
                for j in range(m):
                    half = j % 2
                    if half == 0:
                        pp2 = pp_pool.tile([128, 4 * D], F32)
                        st = stp.tile([128, 2 * 2 * D], R, tag="st")
                    ppv = pp2[:, half * 2 * D : (half + 1) * 2 * D]
                    sl = slice(j * 128, (j + 1) * 128)
                    nc.tensor.matmul(
                        ppv, ft0[:, sl], w0[:],
                        start=True, stop=False, skip_group_check=True,
                    )
                    nc.tensor.matmul(
                        ppv, ft1[:, sl], w1[:],
                        start=False, stop=True, skip_group_check=True,
                    )
                    if half == 1:
                        p3 = pp2[:].rearrange("p (b x) -> p b x", b=2)
                        s3 = st[:].rearrange("p (b x) -> p b x", b=2)
                        nc.scalar.activation(
                            s3[:, :, 0:D], p3[:, :, 0:D],
                            mybir.ActivationFunctionType.Exp,
                        )
                        if SPLIT_MUL and (j // 2) % 2 == 1:
                            # split-mul fast path: ACT stages val in SBUF
                            # fp16 so the DVE multiply runs all-2-byte (2x)
                            v16 = vtp.tile([128, 2 * D], R, tag="v16")
                            v3 = v16[:].rearrange("p (b x) -> p b x", b=2)
                            nc.scalar.copy(v3[:, :, :], p3[:, :, D : 2 * D])
                            nc.vector.tensor_mul(
                                s3[:, :, D : 2 * D], v3[:, :, :],
                                s3[:, :, 0:D],
                            )
                        else:
                            nc.vector.tensor_mul(
                                s3[:, :, D : 2 * D], p3[:, :, D : 2 * D],
                                s3[:, :, 0:D],
                            )
                        # pair-sum the two chunks' [E|VE]; DVE fp16 adds hit
                        # the 2x path (~416ns), GpSimd is 3x slower - keep
                        # pair-sums on DVE
                        pt = ptp.tile([128, 2 * D], R, tag="pt")
                        nc.vector.tensor_add(pt[:], s3[:, 0, :], s3[:, 1, :])
                        pts.append(pt)
                        # interleave the previous group's seg matmul mid-group
                        if j == m // 2 and pending_seg:
                            emit_seg(pending_seg)
                            pending_seg.clear()

                # tree-reduce pair sums into the group accumulator; tree
                # adds go to GpSimd to keep DVE under the PE cadence
                while len(pts) > 1:
                    nxt = []
                    for k in range(0, len(pts) - 1, 2):
                        if len(pts) == 2:
                            dst = accp.tile([128, 2 * D], R, name="acc", tag="acc")
                        else:
                            dst = ptp.tile([128, 2 * D], R, name="pt2", tag="pt2")
                        nc.gpsimd.tensor_add(dst[:], pts[k][:], pts[k + 1][:])
                        nxt.append(dst)
                    if len(pts) % 2 == 1:
                        nxt.append(pts[-1])
                    pts = nxt
                acct = pts[0]
                pending_seg.append(
                    (blk, oht, acct, t == 0, t == gpbs[blk] - 1)
                )
                tg += 1

        emit_seg(pending_seg)
        pending_seg.clear()

    _split_waits(nc)
    return nc


def _pack_host(gid, m):
    """Build per-core lane packing. Returns (assign, gpbs, lanes_per_core)
    where assign[c][k] = block id for core c slot k; lanes_per_core[c] is a
    list (len TG*128) of (graph_local_in_block + blk_slot*GBLK) per lane or
    -1 for pad lanes, plus the node lists."""
    G = BPC * GBLK * N_CORES
    n_blocks = G // GBLK
    counts = np.bincount(gid, minlength=G)
    order = np.argsort(gid, kind="stable")
    g_starts = np.concatenate([[0], np.cumsum(counts)])

    lanes_per_block = np.array(
        [
            int(np.ceil(counts[b * GBLK : (b + 1) * GBLK] / m).sum())
            for b in range(n_blocks)
        ]
    )
    # slot assignment: sort blocks desc, slot k gets ranks k*8..k*8+7
    rank = np.argsort(-lanes_per_block)
    assign = np.zeros((N_CORES, BPC), np.int64)
    gpbs = []
    for k in range(BPC):
        blks = rank[k * N_CORES : (k + 1) * N_CORES]
        for c in range(N_CORES):
            assign[c, k] = blks[c]
        gpbs.append(int(np.ceil(lanes_per_block[blks].max() / 128)))
    return assign, tuple(gpbs), counts, order, g_starts


def kernel(
    f_node,
    key_W,
    key_b,
    value_W,
    value_b,
    gamma,
    beta,
    graph_id,
    num_graphs,
    trace=False,
):
    global LAST_EXEC_TIME_NS, LAST_RESULTS
    f_node = np.asarray(f_node, dtype=np.float32)
    key_W = np.asarray(key_W, dtype=np.float32)
    key_b = np.asarray(key_b, dtype=np.float32)
    value_W = np.asarray(value_W, dtype=np.float32)
    value_b = np.asarray(value_b, dtype=np.float32)
    gamma = np.asarray(gamma, dtype=np.float32)
    beta = np.asarray(beta, dtype=np.float32)
    gid = np.asarray(graph_id).astype(np.int64)
    G = int(num_graphs)
    m = PACK

    L, d = f_node.shape
    assert d == D and G == BPC * GBLK * N_CORES

    assign, gpbs, counts, order, g_starts = _pack_host(gid, m)
    TG = sum(gpbs)
    ncols = TG * m * 128

    # f extended with the pad row: attn(f_pad) == PAD_ATT in every column
    f_pad = np.linalg.solve(
        key_W.astype(np.float64),
        np.full(D, PAD_ATT, np.float64) - key_b.astype(np.float64),
    ).astype(np.float32)
    f_ext = np.concatenate([f_node, f_pad[None, :]], axis=0)
    PADIDX = L

    wcat = np.ascontiguousarray(
        np.concatenate([key_W.T, value_W.T], axis=1)
    ).reshape(2, 128, 2 * D)
    vb_rep = np.ascontiguousarray(np.broadcast_to(value_b, (128, D)))
    eps_rep = np.ascontiguousarray(
        np.broadcast_to(
            (EPS_SOFTMAX / np.exp(key_b)).astype(np.float32), (128, D)
        )
    )
    gm_rep = np.ascontiguousarray(np.broadcast_to(gamma, (128, D)))
    bt_rep = np.ascontiguousarray(np.broadcast_to(beta, (128, D)))
    wcat16 = wcat.astype(np.float16)

    in_maps = []
    ymap = []  # (core, slot) -> block id
    for c in range(N_CORES):
        idx = np.full((TG, m, 128), PADIDX, np.int64)  # [group, chunk, lane]
        ohm = np.zeros((TG, 128, GBLK), np.float16)
        tg0 = 0
        for k in range(BPC):
            b = assign[c, k]
            lane = 0  # lane index within this block's group range
            for gl in range(GBLK):
                g = b * GBLK + gl
                n = counts[g]
                s = g_starts[g]
                nodes = order[s : s + n]
                for ls in range(0, n, m):
                    t = tg0 + lane // 128
                    li = lane % 128
                    seg = nodes[ls : ls + m]
                    idx[t, 0 : len(seg), li] = seg
                    ohm[t, li, gl] = 1.0
                    lane += 1
            tg0 += gpbs[k]
        # fT: [2, 128, ncols]; col = ((t*m)+j)*128 + lane
        cols = idx.reshape(-1)  # [TG*m*128] node ids in col order
        fshard = f_ext[cols]  # [ncols, D] f32
        fT = (
            np.ascontiguousarray(fshard.T.astype(np.float16))
            .reshape(2, 128, ncols)
        )
        in_maps.append(
            {
                "fT": fT,
                "oh": ohm,
                "wcat": wcat16,
                "vbrep": vb_rep,
                "epsrep": eps_rep,
                "gammarep": gm_rep,
                "betarep": bt_rep,
            }
        )
        ymap.append([assign[c, k] for k in range(BPC)])

    key = (gpbs, m)
    if key not in _nc_cache:
        _nc_cache[key] = _build_nc2(gpbs, m)
    nc = _nc_cache[key]

    if trace:
        _install_ntff_hook()
    res = run_bass_kernel_spmd(
        nc, in_maps, core_ids=list(range(N_CORES)), trace=trace
    )
    LAST_EXEC_TIME_NS = res.exec_time_ns
    LAST_RESULTS = res

    out = np.zeros((G, D), np.float32)
    for c in range(N_CORES):
        yc = res.results[c]["y"]
        for k in range(BPC):
            b = ymap[c][k]
            out[b * GBLK : (b + 1) * GBLK] = yc[k * GBLK : (k + 1) * GBLK]
    return out


def _install_ntff_hook():
    import sys, types

    try:
        if "antenv.axon_hooks" in sys.modules:
            return
        mod = types.ModuleType("antenv.axon_hooks")
        state = {"hook": None}
        mod.set_axon_ntff_profile_hook = lambda h: state.__setitem__("hook", h)
        mod.get_axon_ntff_profile_hook = lambda: state["hook"]
        sys.modules["antenv.axon_hooks"] = mod
        import antenv

        antenv.axon_hooks = mod
        from trn_agent_boot.trn_boot import _ntff_profile_via_ctypes

        mod.set_axon_ntff_profile_hook(
            _ntff_profile_via_ctypes("/opt/axon/libaxon_pjrt.so")
        )
    except Exception:
        pass

